# revision 17
# baseline (speedup 1.0000x reference)
"""LIF spike kernel (T=4 scan with threshold reset) on 8 TRN2 NeuronCores.

Recurrence per element (tau=1, thresh=1):
    s_t     = m_{t-1} + x_t
    spike_t = (s_t > 1)           -> output
    m_t     = s_t * (s_t <= 1)    -> threshold reset

Sharding: pure data-parallel over the batch axis (dim 1, 64 -> 8 per core).

v6 design:
  - spikes leave the device as int8 sign planes: q_t = Sign(1 - s_t) in
    {-1,0,1} on the Scalar (ACT) engine (exact at the threshold; s==1 maps
    to q==0 -> no spike, matching the strict >). Host maps q==-1 -> 1.0f.
    Output traffic drops 4x (16 MiB -> 4 MiB per core).
  - DVE runs only the serial recurrence (3 stt resets + 2-3 adds per
    chunk, s_t in place over the per-plane x tiles).
  - the terminal t=3 add (never re-enters the DVE chain) is done by the
    CCE during the x3 load: gpsimd accum-DMA  m3 += x3  (exact f32 add).
    The last chunk keeps it on DVE, split in halves, to shorten the tail.
  - asymmetric chunk widths: a small first chunk so DVE starts ~1 us after
    the first load, a smaller last chunk for the tail.
  - per-plane HWDGE loads on sync; int8 stores on the scalar ring right
    after each sign.
"""

import numpy as np

import concourse.bacc as bacc
import concourse.mybir as mybir
import concourse.tile as tile
from concourse import bass_utils

T = 4
B_FULL = 64
C, H, W = 128, 32, 32
N_CORES = 8
B_LOC = B_FULL // N_CORES            # 8
N = B_LOC * C * H * W                # 1048576 elements per core per timestep
P = 128                              # SBUF partitions
NP = N // P                          # 8192 elements per partition row

_LE = mybir.AluOpType.is_le
_MUL = mybir.AluOpType.mult
_ADD = mybir.AluOpType.add
_F32 = mybir.dt.float32
_I8 = mybir.dt.int8
_SIGN = mybir.ActivationFunctionType.Sign

# chunk widths (elements per partition); sum must be NP. Big first chunk =
# natural load headroom (loads and DVE are rate-matched, so the pipeline
# needs a few us of buffered input to absorb the stores' wire-steal);
# tiny last chunk = short tail.
FS = [1024, 2048, 2048, 2048, 1024]
assert sum(FS) == NP

_nc_cache = None


def _build(fs=tuple(FS), bufs=4):
    nc = bacc.Bacc(
        "TRN2",
        target_bir_lowering=False,
        debug=False,
        enable_asserts=False,
    )
    x_d = nc.dram_tensor("x", [T, N], _F32, kind="ExternalInput").ap()
    y_d = nc.dram_tensor("y", [T, N], _I8, kind="ExternalOutput").ap()

    def xsl(t, base, f):
        return x_d[t, P * base : P * (base + f)].rearrange("(p f) -> p f", p=P)

    def ysl(t, base, f):
        return y_d[t, P * base : P * (base + f)].rearrange("(p f) -> p f", p=P)

    nchunk = len(fs)
    with tile.TileContext(nc) as tc:
        with (
            tc.tile_pool(name="xx", bufs=bufs) as xp,
            tc.tile_pool(name="mm", bufs=3) as mp,
            tc.tile_pool(name="qq", bufs=bufs) as qp,
        ):
            base = 0
            for j, F in enumerate(fs):
                last = j == nchunk - 1
                sl = []
                for t in range(T):
                    xt = xp.tile([P, F], _F32, tag=f"x{t}", name=f"x{t}_{j}")
                    if j == 0 and t == 0:
                        # split the very first load so DVE starts after
                        # half a plane instead of a full one
                        h = F // 2
                        src = xsl(t, base, F)
                        nc.sync.dma_start(xt[:, :h], src[:, :h])
                        nc.sync.dma_start(xt[:, h:], src[:, h:])
                    else:
                        nc.sync.dma_start(xt[:], xsl(t, base, F))
                    sl.append(xt[:])
                m = mp.tile([P, F], _F32, tag="m", name=f"m_{j}")
                ydst = [ysl(t, base, F) for t in range(T)]

                v = nc.vector

                def sign_store(src, t, off=0, w=F, k=""):
                    q = qp.tile([P, w], _I8, tag=f"q{t}{k}", name=f"q{t}{k}_{j}")
                    nc.scalar.activation(q[:], src, _SIGN, bias=1.0, scale=-1.0)
                    nc.scalar.dma_start(ydst[t][:, off : off + w], q[:])

                # t = 0
                if j == 0:
                    h = F // 2
                    for off, k in ((0, "a"), (h, "b")):
                        s = sl[0][:, off : off + h]
                        v.scalar_tensor_tensor(
                            m[:, off : off + h], s, 1.0, s, _LE, _MUL
                        )
                        sign_store(s, 0, off, h, k)
                else:
                    v.scalar_tensor_tensor(m[:], sl[0], 1.0, sl[0], _LE, _MUL)
                    sign_store(sl[0], 0)
                # t = 1, 2
                for t in (1, 2):
                    v.tensor_tensor(sl[t], m[:], sl[t], _ADD)
                    v.scalar_tensor_tensor(m[:], sl[t], 1.0, sl[t], _LE, _MUL)
                    sign_store(sl[t], t)
                # t = 3 (terminal add, no reset)
                v.tensor_tensor(sl[3], m[:], sl[3], _ADD)
                sign_store(sl[3], 3)
                base += F

    nc.compile()
    return nc


def _get_nc():
    global _nc_cache
    if _nc_cache is None:
        _nc_cache = _build()
    return _nc_cache


def _run(x, **spmd_kwargs):
    x = np.asarray(x, dtype=np.float32)
    assert x.shape == (T, B_FULL, C, H, W), x.shape
    in_maps = [
        {
            "x": np.ascontiguousarray(
                x[:, c * B_LOC : (c + 1) * B_LOC]
            ).reshape(T, N)
        }
        for c in range(N_CORES)
    ]
    res = bass_utils.run_bass_kernel_spmd(
        _get_nc(), in_maps, core_ids=list(range(N_CORES)), **spmd_kwargs
    )
    out = np.empty((T, B_FULL, C, H, W), dtype=np.float32)
    for c in range(N_CORES):
        y = res.results[c]["y"]
        sp = (y.reshape(T, N) == -1).astype(np.float32)
        out[:, c * B_LOC : (c + 1) * B_LOC] = sp.reshape(T, B_LOC, C, H, W)
    return out, res


def kernel(x):
    out, _ = _run(x)
    return out


# revision 19
# speedup vs baseline: 1.0017x; 1.0017x over previous
"""LIF spike kernel (T=4 scan with threshold reset) on 8 TRN2 NeuronCores.

Recurrence per element (tau=1, thresh=1):
    s_t     = m_{t-1} + x_t
    spike_t = (s_t > 1)           -> output
    m_t     = s_t * (s_t <= 1)    -> threshold reset

Sharding: pure data-parallel over the batch axis (dim 1, 64 -> 8 per core).

Final design (HW exec ~74 us/core vs ~110 us for the f32-out all-DVE
baseline):
  - spikes leave the device as int8 sign planes: q_t = Sign(1 - s_t) in
    {-1,0,1} on the otherwise-idle Scalar (ACT) engine (exact at the
    threshold: Sign is not LUT-interpolated, and s==1 maps to q==0 -> no
    spike, matching the strict >). Host maps q==-1 -> 1.0f for free.
    Output traffic drops 4x (16 MiB -> 4 MiB per core).
  - DVE runs only the serial recurrence: per chunk 3 stt resets
    (m = s*(s<=1), one fused instruction each) + 3 tensor_tensor adds,
    with s_t computed in place over the per-plane x tiles so ACT reads
    never block the DVE chain. This is the hard wall: 6 fp32 two-tensor
    ops/element-row = ~56 us of DVE at 128 lanes x 0.96 GHz.
    (Measured dead ends: GPSIMD tensor ops steal a DVE SBUF port and
    inflate DVE ops ~30%; CCE accum-DMA adds run at half wire rate and
    starve the loads; TensorE matmul-adds need SBUF sources + PSUM dest,
    which re-triggers the stt both-PSUM-source restriction.)
  - per-plane HWDGE loads on the sync ring (DVE's first op waits on one
    1 MB plane, not a whole chunk; the first plane is further split in
    halves); int8 stores ride the scalar ring right after each sign.
  - asymmetric chunk widths: big first chunks leave the load stream a few
    us of headroom over the rate-matched DVE (absorbing the stores'
    wire-steal); a small last chunk keeps the tail short. bufs=4 so tile
    slot recycling (released only after ACT's sign) never throttles loads.
"""

import numpy as np

import concourse.bacc as bacc
import concourse.mybir as mybir
import concourse.tile as tile
from concourse import bass_utils

T = 4
B_FULL = 64
C, H, W = 128, 32, 32
N_CORES = 8
B_LOC = B_FULL // N_CORES            # 8
N = B_LOC * C * H * W                # 1048576 elements per core per timestep
P = 128                              # SBUF partitions
NP = N // P                          # 8192 elements per partition row

_LE = mybir.AluOpType.is_le
_MUL = mybir.AluOpType.mult
_ADD = mybir.AluOpType.add
_F32 = mybir.dt.float32
_I8 = mybir.dt.int8
_SIGN = mybir.ActivationFunctionType.Sign

# chunk widths (elements per partition); sum must be NP. Big first chunk =
# natural load headroom (loads and DVE are rate-matched, so the pipeline
# needs a few us of buffered input to absorb the stores' wire-steal);
# tiny last chunk = short tail.
FS = [2048, 2048, 2048, 1536, 512]
assert sum(FS) == NP

_nc_cache = None


def _build(fs=tuple(FS), bufs=4):
    nc = bacc.Bacc(
        "TRN2",
        target_bir_lowering=False,
        debug=False,
        enable_asserts=False,
    )
    x_d = nc.dram_tensor("x", [T, N], _F32, kind="ExternalInput").ap()
    y_d = nc.dram_tensor("y", [T, N], _I8, kind="ExternalOutput").ap()

    def xsl(t, base, f):
        return x_d[t, P * base : P * (base + f)].rearrange("(p f) -> p f", p=P)

    def ysl(t, base, f):
        return y_d[t, P * base : P * (base + f)].rearrange("(p f) -> p f", p=P)

    nchunk = len(fs)
    with tile.TileContext(nc) as tc:
        with (
            tc.tile_pool(name="xx", bufs=bufs) as xp,
            tc.tile_pool(name="mm", bufs=3) as mp,
            tc.tile_pool(name="qq", bufs=bufs) as qp,
        ):
            base = 0
            for j, F in enumerate(fs):
                last = j == nchunk - 1
                sl = []
                for t in range(T):
                    xt = xp.tile([P, F], _F32, tag=f"x{t}", name=f"x{t}_{j}")
                    if j == 0 and t == 0:
                        # split the very first load so DVE starts after
                        # half a plane instead of a full one
                        h = F // 2
                        src = xsl(t, base, F)
                        nc.sync.dma_start(xt[:, :h], src[:, :h])
                        nc.sync.dma_start(xt[:, h:], src[:, h:])
                    else:
                        nc.sync.dma_start(xt[:], xsl(t, base, F))
                    sl.append(xt[:])
                m = mp.tile([P, F], _F32, tag="m", name=f"m_{j}")
                ydst = [ysl(t, base, F) for t in range(T)]

                v = nc.vector

                def sign_store(src, t, off=0, w=F, k=""):
                    q = qp.tile([P, w], _I8, tag=f"q{t}{k}", name=f"q{t}{k}_{j}")
                    nc.scalar.activation(q[:], src, _SIGN, bias=1.0, scale=-1.0)
                    nc.scalar.dma_start(ydst[t][:, off : off + w], q[:])

                # t = 0
                if j == 0:
                    h = F // 2
                    for off, k in ((0, "a"), (h, "b")):
                        s = sl[0][:, off : off + h]
                        v.scalar_tensor_tensor(
                            m[:, off : off + h], s, 1.0, s, _LE, _MUL
                        )
                        sign_store(s, 0, off, h, k)
                else:
                    v.scalar_tensor_tensor(m[:], sl[0], 1.0, sl[0], _LE, _MUL)
                    sign_store(sl[0], 0)
                # t = 1, 2
                for t in (1, 2):
                    v.tensor_tensor(sl[t], m[:], sl[t], _ADD)
                    v.scalar_tensor_tensor(m[:], sl[t], 1.0, sl[t], _LE, _MUL)
                    sign_store(sl[t], t)
                # t = 3 (terminal add, no reset)
                v.tensor_tensor(sl[3], m[:], sl[3], _ADD)
                sign_store(sl[3], 3)
                base += F

    nc.compile()
    return nc


def _get_nc():
    global _nc_cache
    if _nc_cache is None:
        _nc_cache = _build()
    return _nc_cache


def _run(x, **spmd_kwargs):
    x = np.asarray(x, dtype=np.float32)
    assert x.shape == (T, B_FULL, C, H, W), x.shape
    in_maps = [
        {
            "x": np.ascontiguousarray(
                x[:, c * B_LOC : (c + 1) * B_LOC]
            ).reshape(T, N)
        }
        for c in range(N_CORES)
    ]
    res = bass_utils.run_bass_kernel_spmd(
        _get_nc(), in_maps, core_ids=list(range(N_CORES)), **spmd_kwargs
    )
    out = np.empty((T, B_FULL, C, H, W), dtype=np.float32)
    for c in range(N_CORES):
        y = res.results[c]["y"]
        sp = (y.reshape(T, N) == -1).astype(np.float32)
        out[:, c * B_LOC : (c + 1) * B_LOC] = sp.reshape(T, B_LOC, C, H, W)
    return out, res


def kernel(x):
    out, _ = _run(x)
    return out
